# revision 12
# baseline (speedup 1.0000x reference)
"""GATv2 (2 layers, N=50000, E=600000, C=128) on 8 trn2 NeuronCores.

Strategy:
  - Append self-loops, sort edges by dst. Shard edges across 8 cores by
    contiguous dst node ranges (6272 nodes/core) so edge-softmax segment
    statistics stay core-local (no collectives).
  - Per core: 49 blocks x 128 dst nodes. Per block, edges are packed into
    K_A*128 "A" slots (src < 32768) and K_B*128 "B" slots (src >= 32768) so
    the int16-indexed dma_gather custom instruction can fetch xl[src] rows
    from the (bf16) projection table in two batched calls per block.
  - xr[dst] is expanded on the TensorEngine: xr_exp = AT.T @ xr_block, with
    the one-hot AT supplied by the host; the rank-1 edge-attr term ea (x) We
    accumulates into the same PSUM via a K=1 matmul.
  - logit_e = sum_c att_c * leaky_relu(z_ec) via scalar-engine Relu +
    fused tensor_tensor_reduce (leaky_relu = 0.2 z + 0.8 relu(z)).
  - Edge softmax without max subtraction (logits are O(10), fp32-safe);
    unnormalized aggregation u = sum ex*xl[src] and den = sum ex via one-hot
    matmuls accumulated in PSUM per block; out = u/(den+eps) + bias.
  - Projections (xl = h @ Wl + bl etc.) run replicated per core from a
    host-transposed input; layer 2 is a second launch of the same NEFF.
"""
import sys
sys.path.insert(0, "/opt/trn_rl_repo")

import numpy as np
import ml_dtypes

BF16 = ml_dtypes.bfloat16

# ---------------------------------------------------------------- config
N = 50000
E = 600000
C = 128
N_CORES = 8
BLK = 128                     # dst nodes per block
BLOCKS = 49                   # blocks per core
LOCAL = BLOCKS * BLK          # 6272 nodes per core
N_PAD = N_CORES * LOCAL       # 50176
SPLIT = 32768                 # A/B table split for int16 dma_gather
EPS = 1e-16
SLOPE = 0.2


# ---------------------------------------------------------------- host prep
def _pack_graph(src, dst, ea, n_pad=N_PAD, n_cores=N_CORES, blocks=BLOCKS,
                split=SPLIT):
    """Sort edges by dst, shard by dst range, pack into per-block A/B slots.

    Returns dict with per-core arrays and K_A/K_B.
    """
    local = blocks * BLK
    order = np.argsort(dst, kind="stable")
    src_s = src[order].astype(np.int64)
    dst_s = dst[order].astype(np.int64)
    ea_s = ea[order].astype(np.float32)

    is_a = src_s < split
    gblk = dst_s // BLK                      # global block id
    n_gblk = n_cores * blocks

    # count A/B edges per global block
    cnt_a = np.bincount(gblk[is_a], minlength=n_gblk)
    cnt_b = np.bincount(gblk[~is_a], minlength=n_gblk)
    K_A = int(np.ceil(cnt_a.max() / BLK)) if cnt_a.max() else 1
    K_B = int(np.ceil(cnt_b.max() / BLK)) if cnt_b.max() else 1

    SA, SB = K_A * BLK, K_B * BLK
    K = K_A + K_B

    # slot index within the block for every edge (A edges fill 0..; B edges
    # fill SA..). stable order within block.
    # rank of edge within (block, is_a) group:
    keys = gblk * 2 + (~is_a)
    sort2 = np.argsort(keys, kind="stable")
    inv = np.empty_like(sort2)
    inv[sort2] = np.arange(len(sort2))
    grp_start = np.zeros(2 * n_gblk, np.int64)
    cnt = np.bincount(keys, minlength=2 * n_gblk)
    grp_start[1:] = np.cumsum(cnt)[:-1]
    rank = inv - grp_start[keys]
    slot = np.where(is_a, rank, SA + rank)          # slot within block

    S = SA + SB
    n_slots = n_gblk * S
    # flat slot id per edge
    flat = gblk * S + slot

    # build per-slot arrays (defaults: idx 0, dst_local -1 -> A row zero)
    slot_src = np.zeros(n_slots, np.int64)
    slot_m = np.full(n_slots, -1, np.int64)          # local dst (0..127), -1 pad
    slot_ea = np.zeros(n_slots, np.float32)
    slot_src[flat] = src_s
    slot_m[flat] = dst_s - gblk * BLK
    slot_ea[flat] = ea_s[:, 0] if ea_s.ndim == 2 else ea_s

    slot_src = slot_src.reshape(n_cores, blocks, S)
    slot_m = slot_m.reshape(n_cores, blocks, S)
    slot_ea = slot_ea.reshape(n_cores, blocks, S)

    # gather index arrays, int16, wrapped [16, n/16] replicated to 128 parts
    def wrap_idx(ix, nidx):
        # ix: [cores, blocks, nidx] values already offset for table half
        w = ix.reshape(n_cores, blocks, nidx // 16, 16)
        w = np.ascontiguousarray(w.transpose(0, 1, 3, 2)).astype(np.int16)
        # replicate across 8 groups of 16 partitions
        return np.tile(w, (1, 1, 8, 1))               # [cores, blocks, 128, nidx//16]

    idx_a = wrap_idx(slot_src[:, :, :SA], SA)
    idx_b = wrap_idx(np.maximum(slot_src[:, :, SA:] - split, 0), SB)

    # one-hot A [slots->tile(slot//128) partition(slot%128), m] and AT
    sm = slot_m.reshape(n_cores, blocks, K, BLK)      # [.., tile, partition]
    eye = np.eye(BLK, dtype=np.float32)
    zero_row = np.zeros((1, BLK), np.float32)
    lut = np.concatenate([eye, zero_row], 0)          # row 128 = zeros for pad
    smi = np.where(sm < 0, BLK, sm)
    A_h = lut[smi]                                    # [cores, blocks, K, 128, 128]
    AT_h = np.ascontiguousarray(A_h.transpose(0, 1, 2, 4, 3))
    A_h = A_h.astype(BF16)
    AT_h = AT_h.astype(np.float32)

    ea_h = slot_ea.reshape(n_cores, blocks, K, 1, BLK).astype(np.float32)

    return dict(K_A=K_A, K_B=K_B, K=K, idx_a=idx_a, idx_b=idx_b,
                A_h=A_h, AT_h=AT_h, ea_h=ea_h)


# ---------------------------------------------------------------- bass build
def build_nc(K_A, K_B, n_pad=N_PAD, blocks=BLOCKS, split=SPLIT,
             use_lrelu=False, copy_split=2, stage="full"):
    import concourse.bacc as bacc
    import concourse.mybir as mybir
    from concourse.tile import TileContext

    f32 = mybir.dt.float32
    bf16 = mybir.dt.bfloat16
    i16 = mybir.dt.int16
    AF = mybir.ActivationFunctionType
    OP = mybir.AluOpType

    K = K_A + K_B
    SA, SB = K_A * BLK, K_B * BLK
    local = blocks * BLK
    ntile_full = n_pad // BLK
    hi_rows = n_pad - split

    nc = bacc.Bacc(None, target_bir_lowering=False, debug=False)

    # inputs
    xT = nc.dram_tensor("xT", [C, n_pad], f32, kind="ExternalInput")
    xTloc = nc.dram_tensor("xTloc", [C, local], f32, kind="ExternalInput")
    Wl = nc.dram_tensor("Wl", [C, C], f32, kind="ExternalInput")
    Wr = nc.dram_tensor("Wr", [C, C], f32, kind="ExternalInput")
    blr = nc.dram_tensor("blr", [1, C], f32, kind="ExternalInput")
    brr = nc.dram_tensor("brr", [1, C], f32, kind="ExternalInput")
    Wer = nc.dram_tensor("Wer", [1, C], f32, kind="ExternalInput")
    attb = nc.dram_tensor("attb", [BLK, C], f32, kind="ExternalInput")
    biasb = nc.dram_tensor("biasb", [BLK, C], f32, kind="ExternalInput")
    idxA = nc.dram_tensor("idxA", [blocks, BLK, SA // 16], i16, kind="ExternalInput")
    idxB = nc.dram_tensor("idxB", [blocks, BLK, SB // 16], i16, kind="ExternalInput")
    A_h = nc.dram_tensor("A_h", [blocks * K * BLK, BLK], bf16, kind="ExternalInput")
    AT_h = nc.dram_tensor("AT_h", [blocks * K * BLK, BLK], f32, kind="ExternalInput")
    ea_h = nc.dram_tensor("ea_h", [blocks * K, BLK], f32, kind="ExternalInput")
    out_d = nc.dram_tensor("out_d", [local, C], f32, kind="ExternalOutput")

    with TileContext(nc) as tc:
        with (
            tc.tile_pool(name="dram", bufs=1, space="DRAM") as dpool,
            tc.tile_pool(name="consts", bufs=1) as cpool,
            tc.tile_pool(name="proj", bufs=4) as ppool,
            tc.tile_pool(name="gath", bufs=2) as gpool,
            tc.tile_pool(name="work", bufs=4) as wpool,
            tc.tile_pool(name="small", bufs=6) as spool,
            tc.tile_pool(name="psz", bufs=2, space="PSUM") as zpool,
            tc.tile_pool(name="psu", bufs=2, space="PSUM") as upool,
            tc.tile_pool(name="psd", bufs=2, space="PSUM") as dnpool,
            tc.tile_pool(name="psp", bufs=2, space="PSUM") as qpool,
        ):
            xl_t = dpool.tile([n_pad, C], f32)       # xl projection table
            xr_t = dpool.tile([local, C], f32)       # local xr rows

            # consts to SBUF
            Wl_s = cpool.tile([C, C], f32)
            Wr_s = cpool.tile([C, C], f32)
            bl_s = cpool.tile([1, C], f32)
            br_s = cpool.tile([1, C], f32)
            We_s = cpool.tile([1, C], f32)
            att_s = cpool.tile([BLK, C], f32)
            bias_s = cpool.tile([BLK, C], f32)
            ones_r = cpool.tile([1, BLK], f32)
            ones_c = cpool.tile([BLK, 1], f32)
            nc.sync.dma_start(out=Wl_s[:], in_=Wl.ap())
            nc.sync.dma_start(out=Wr_s[:], in_=Wr.ap())
            nc.sync.dma_start(out=bl_s[:], in_=blr.ap())
            nc.sync.dma_start(out=br_s[:], in_=brr.ap())
            nc.sync.dma_start(out=We_s[:], in_=Wer.ap())
            nc.sync.dma_start(out=att_s[:], in_=attb.ap())
            nc.sync.dma_start(out=bias_s[:], in_=biasb.ap())
            nc.vector.memset(ones_r[:], 1.0)
            nc.vector.memset(ones_c[:], 1.0)

            # ---------------- projection: xl table (all nodes)
            def proj(n_tiles, src_ap, W_s, b_s, table, ci):
                for nblk in range(n_tiles):
                    xt = ppool.tile([C, BLK], f32, tag="xt")
                    nc.sync.dma_start(
                        out=xt[:], in_=src_ap[:, nblk * BLK:(nblk + 1) * BLK])
                    ps = qpool.tile([BLK, C], mybir.dt.float32, tag="psp")
                    nc.tensor.matmul(out=ps[:], lhsT=xt[:], rhs=W_s[:],
                                     start=True, stop=False)
                    nc.tensor.matmul(out=ps[:], lhsT=ones_r[:], rhs=b_s[:],
                                     start=False, stop=True)
                    xo = ppool.tile([BLK, C], f32, tag="xo")
                    if ci % copy_split == 0:
                        nc.vector.tensor_copy(out=xo[:], in_=ps[:])
                    else:
                        nc.scalar.copy(out=xo[:], in_=ps[:])
                    ci += 1
                    nc.sync.dma_start(
                        out=table[nblk * BLK:(nblk + 1) * BLK, :], in_=xo[:])
                return ci

            ci = proj(ntile_full, xT.ap(), Wl_s, bl_s, xl_t, 0)
            proj(blocks, xTloc.ap(), Wr_s, br_s, xr_t, ci)

            # ---------------- edge phase
            for b in range(blocks):
                if stage == "proj":
                    ob = wpool.tile([BLK, C], f32, tag="ot2")
                    nc.vector.tensor_copy(out=ob[:], in_=bias_s[:])
                    nc.sync.dma_start(
                        out=out_d.ap()[b * BLK:(b + 1) * BLK, :], in_=ob[:])
                    continue
                xrb = wpool.tile([BLK, C], f32, tag="xrb")
                nc.sync.dma_start(out=xrb[:],
                                  in_=xr_t[b * BLK:(b + 1) * BLK, :])
                ia = wpool.tile([BLK, SA // 16], i16, tag="ia")
                nc.sync.dma_start(out=ia[:], in_=idxA.ap()[b])
                ib = wpool.tile([BLK, SB // 16], i16, tag="ib")
                nc.sync.dma_start(out=ib[:], in_=idxB.ap()[b])

                gl = gpool.tile([BLK, K * C], f32, tag="gl")
                nc.gpsimd.dma_gather(
                    out_ap=gl[:, 0:SA // BLK * C].rearrange(
                        "p (a c) -> p a c", c=C),
                    in_ap=xl_t[0:split, :],
                    idxs_ap=ia[:],
                    num_idxs=SA, num_idxs_reg=SA, elem_size=C,
                    single_packet=(SA <= 1024))
                nc.gpsimd.dma_gather(
                    out_ap=gl[:, SA // BLK * C:].rearrange(
                        "p (a c) -> p a c", c=C),
                    in_ap=xl_t[split:n_pad, :],
                    idxs_ap=ib[:],
                    num_idxs=SB, num_idxs_reg=SB, elem_size=C,
                    single_packet=(SB <= 1024))

                if stage == "gather":
                    ob = wpool.tile([BLK, C], f32, tag="ot2")
                    nc.vector.tensor_copy(out=ob[:], in_=gl[:, 0:C])
                    nc.sync.dma_start(
                        out=out_d.ap()[b * BLK:(b + 1) * BLK, :], in_=ob[:])
                    continue

                pu = pd = None
                if stage in ("agg", "full"):
                    pu = upool.tile([BLK, C], mybir.dt.float32, tag="pu")
                    pd = dnpool.tile([BLK, 1], mybir.dt.float32, tag="pd")
                for k in range(K):
                    row0 = (b * K + k) * BLK
                    At = wpool.tile([BLK, BLK], bf16, tag="At")
                    nc.sync.dma_start(out=At[:], in_=A_h.ap()[row0:row0 + BLK, :])
                    ATt = wpool.tile([BLK, BLK], f32, tag="ATt")
                    nc.sync.dma_start(out=ATt[:], in_=AT_h.ap()[row0:row0 + BLK, :])
                    ear = spool.tile([1, BLK], f32, tag="ear")
                    nc.sync.dma_start(out=ear[:], in_=ea_h.ap()[b * K + k, :][None, :])

                    pz = zpool.tile([BLK, C], mybir.dt.float32, tag="pz")
                    nc.tensor.matmul(out=pz[:], lhsT=ATt[:], rhs=xrb[:],
                                     start=True, stop=False)
                    nc.tensor.matmul(out=pz[:], lhsT=ear[:], rhs=We_s[:],
                                     start=False, stop=True)

                    glk = gl[:, k * C:(k + 1) * C]
                    z = wpool.tile([BLK, C], f32, tag="z")
                    nc.vector.tensor_add(out=z[:], in0=glk, in1=pz[:])

                    if stage == "zadd":
                        if k == K - 1:
                            nc.sync.dma_start(
                                out=out_d.ap()[b * BLK:(b + 1) * BLK, :], in_=z[:])
                        continue

                    scr = wpool.tile([BLK, C], f32, tag="scr")
                    ex = spool.tile([BLK, 1], f32, tag="ex")
                    if True:
                        # logit = att . lrelu(z) = 0.2*(att.z) + 0.8*(att.relu(z))
                        q1 = spool.tile([BLK, 1], f32, tag="q1")
                        nc.vector.tensor_mul(out=scr[:], in0=z[:], in1=att_s[:])
                        nc.vector.tensor_reduce(out=q1[:], in_=scr[:],
                                                axis=mybir.AxisListType.X,
                                                op=OP.add)
                        r = wpool.tile([BLK, C], f32, tag="r")
                        nc.scalar.activation(out=r[:], in_=z[:], func=AF.Relu)
                        q2 = spool.tile([BLK, 1], f32, tag="q2")
                        scr2 = wpool.tile([BLK, C], f32, tag="scr2")
                        nc.vector.tensor_mul(out=scr2[:], in0=r[:], in1=att_s[:])
                        nc.vector.tensor_reduce(out=q2[:], in_=scr2[:],
                                                axis=mybir.AxisListType.X,
                                                op=OP.add)
                        b2 = spool.tile([BLK, 1], f32, tag="b2")
                        nc.scalar.mul(out=b2[:], in_=q1[:], mul=SLOPE)
                        nc.scalar.activation(out=ex[:], in_=q2[:], func=AF.Exp,
                                             bias=b2[:], scale=1.0 - SLOPE)

                    if stage in ("logit", "ttr1", "relu"):
                        if k == K - 1:
                            ob = wpool.tile([BLK, C], f32, tag="ot2")
                            nc.vector.tensor_scalar_mul(out=ob[:], in0=scr[:],
                                                        scalar1=ex[:])
                            nc.sync.dma_start(
                                out=out_d.ap()[b * BLK:(b + 1) * BLK, :], in_=ob[:])
                        continue

                    Ap = wpool.tile([BLK, BLK], f32, tag="Ap")
                    nc.vector.tensor_scalar_mul(out=Ap[:], in0=At[:], scalar1=ex[:])

                    nc.tensor.matmul(out=pu[:, 0:C], lhsT=Ap[:], rhs=glk,
                                     start=(k == 0), stop=(k == K - 1))
                    nc.tensor.matmul(out=pd[:], lhsT=Ap[:], rhs=ones_c[:],
                                     start=(k == 0), stop=(k == K - 1))

                if stage in ("zadd", "logit", "ttr1", "relu"):
                    continue
                if stage == "agg":
                    ob = wpool.tile([BLK, C], f32, tag="ot2")
                    nc.vector.tensor_copy(out=ob[:], in_=pu[:, 0:C])
                    nc.sync.dma_start(
                        out=out_d.ap()[b * BLK:(b + 1) * BLK, :], in_=ob[:])
                    continue

                den = spool.tile([BLK, 1], f32, tag="den")
                nc.vector.tensor_scalar_add(out=den[:], in0=pd[:], scalar1=EPS)
                rec = spool.tile([BLK, 1], f32, tag="rec")
                nc.vector.reciprocal(out=rec[:], in_=den[:])
                ot = wpool.tile([BLK, C], f32, tag="ot")
                nc.vector.tensor_scalar_mul(out=ot[:], in0=pu[:, 0:C],
                                            scalar1=rec[:])
                ot2 = wpool.tile([BLK, C], f32, tag="ot2")
                nc.vector.tensor_add(out=ot2[:], in0=ot[:], in1=bias_s[:])
                nc.sync.dma_start(out=out_d.ap()[b * BLK:(b + 1) * BLK, :],
                                  in_=ot2[:])
    nc.compile()
    return nc


# ---------------------------------------------------------------- runner
class _SpmdRunner:
    def __init__(self, nc, n_cores):
        import jax
        from jax.sharding import Mesh, PartitionSpec
        from jax.experimental.shard_map import shard_map
        import concourse.mybir as mybir
        from concourse.bass2jax import (_bass_exec_p, install_neuronx_cc_hook,
                                        partition_id_tensor)
        install_neuronx_cc_hook()
        self.jax = jax
        self.n_cores = n_cores
        partition_name = (nc.partition_id_tensor.name
                          if nc.partition_id_tensor else None)
        in_names, out_names, out_avals = [], [], []
        for alloc in nc.m.functions[0].allocations:
            if not isinstance(alloc, mybir.MemoryLocationSet):
                continue
            name = alloc.memorylocations[0].name
            if alloc.kind == "ExternalInput":
                if name != partition_name:
                    in_names.append(name)
            elif alloc.kind == "ExternalOutput":
                out_names.append(name)
                out_avals.append(jax.core.ShapedArray(
                    tuple(alloc.tensor_shape), mybir.dt.np(alloc.dtype)))
        self.in_names, self.out_names, self.out_avals = (in_names, out_names,
                                                         out_avals)

        def _body(*args):
            operands = list(args)
            if partition_name is not None:
                operands.append(partition_id_tensor())
            outs = _bass_exec_p.bind(
                *operands,
                out_avals=tuple(out_avals),
                in_names=tuple(in_names + out_names
                               + ([partition_name] if partition_name else [])),
                out_names=tuple(out_names),
                lowering_input_output_aliases=(),
                sim_require_finite=False,
                sim_require_nnan=False,
                nc=nc,
            )
            return tuple(outs)

        devices = jax.devices()[:n_cores]
        assert len(devices) == n_cores
        self.mesh = Mesh(np.asarray(devices), ("core",))
        n_in = len(in_names) + len(out_names)
        self._fn = jax.jit(
            shard_map(_body, mesh=self.mesh,
                      in_specs=(PartitionSpec("core"),) * n_in,
                      out_specs=(PartitionSpec("core"),) * len(out_names),
                      check_rep=False),
            donate_argnums=tuple(range(len(in_names), n_in)),
            keep_unused=True,
        )

    def run(self, per_core_named, device_cache=None):
        """per_core_named: dict name -> list of n_cores np arrays (or a single
        array to replicate). Returns list of per-core output dicts."""
        import jax
        from jax.sharding import NamedSharding, PartitionSpec
        shard = NamedSharding(self.mesh, PartitionSpec("core"))
        packed = []
        for name in self.in_names:
            v = per_core_named[name]
            key = id(v) if device_cache is not None else None
            if key is not None and key in device_cache:
                packed.append(device_cache[key])
                continue
            if isinstance(v, list):
                g = np.concatenate([np.asarray(a) for a in v], axis=0)
            else:
                g = np.concatenate([np.asarray(v)] * self.n_cores, axis=0)
            g = jax.device_put(g, shard)
            if key is not None:
                device_cache[key] = g
            packed.append(g)
        zouts = [np.zeros((self.n_cores * a.shape[0], *a.shape[1:]), a.dtype)
                 for a in self.out_avals]
        outs = self._fn(*packed, *zouts)
        outs = [np.asarray(o) for o in outs]
        return [
            {name: outs[i].reshape(self.n_cores, *self.out_avals[i].shape)[c]
             for i, name in enumerate(self.out_names)}
            for c in range(self.n_cores)]


# ---------------------------------------------------------------- kernel
_CACHE = {}


def _get_runner(K_A, K_B, use_lrelu):
    key = (K_A, K_B, use_lrelu)
    if key not in _CACHE:
        nc = build_nc(K_A, K_B, use_lrelu=use_lrelu)
        _CACHE[key] = _SpmdRunner(nc, N_CORES)
    return _CACHE[key]


def _graph_inputs(g):
    """Per-core graph arrays (layer-independent; build once, reuse)."""
    return {
        "idxA": [g["idx_a"][c] for c in range(N_CORES)],
        "idxB": [g["idx_b"][c] for c in range(N_CORES)],
        "A_h": [g["A_h"][c].reshape(-1, BLK) for c in range(N_CORES)],
        "AT_h": [g["AT_h"][c].reshape(-1, BLK) for c in range(N_CORES)],
        "ea_h": [g["ea_h"][c].reshape(-1, BLK) for c in range(N_CORES)],
    }


def _layer_inputs(h, params_layer, graph_inputs):
    """h: [N_PAD, C] f32. Returns dict of per-core input arrays."""
    Wl, bl, Wr, br, We, att, bias = params_layer
    xT = np.ascontiguousarray(h.T)                     # [C, N_PAD]
    att_b = np.tile(np.asarray(att, np.float32)[None, :], (BLK, 1))
    bias_b = np.tile(np.asarray(bias, np.float32)[None, :], (BLK, 1))
    d = {
        "xT": xT,
        "xTloc": [np.ascontiguousarray(xT[:, c * LOCAL:(c + 1) * LOCAL])
                  for c in range(N_CORES)],
        "Wl": np.asarray(Wl, np.float32),
        "Wr": np.asarray(Wr, np.float32),
        "blr": np.asarray(bl, np.float32)[None, :],
        "brr": np.asarray(br, np.float32)[None, :],
        "Wer": np.asarray(We, np.float32).reshape(1, C),
        "attb": att_b,
        "biasb": bias_b,
    }
    d.update(graph_inputs)
    return d


def kernel(x, edge_index, edge_attr, params):
    x = np.asarray(x, np.float32)
    edge_index = np.asarray(edge_index)
    edge_attr = np.asarray(edge_attr, np.float32)
    params = [[np.asarray(p) for p in layer] for layer in params]

    n = x.shape[0]
    loops = np.arange(n, dtype=np.int64)
    src = np.concatenate([edge_index[0], loops])
    dst = np.concatenate([edge_index[1], loops])
    ea = np.concatenate([edge_attr[:, 0], np.ones(n, np.float32)])

    g = _pack_graph(src, dst, ea)
    runner = _get_runner(g["K_A"], g["K_B"], use_lrelu=False)

    h = np.zeros((N_PAD, C), np.float32)
    h[:n] = x
    dev_cache = {}
    gi = _graph_inputs(g)
    for li, p in enumerate(params):
        inputs = _layer_inputs(h, p, gi)
        # graph arrays are identical across layers -> cache device copies
        res = runner.run(inputs, device_cache=dev_cache)
        h = np.concatenate([res[c]["out_d"] for c in range(N_CORES)], axis=0)
        h[n:] = 0.0
    return h[:n]
